# revision 3
# baseline (speedup 1.0000x reference)
"""PSLoRA linear layer on 8 Trainium2 NeuronCores (Bass/Tile, bf16).

out[b] = x[b] @ W.T + bias + 0.5 * (x[b] @ lora_A[idx[b]]) @ lora_B.T

Sharding: data-parallel over batch (B=8 -> one batch element per core).
W / lora params are replicated; the per-core lora_A gather happens on host
(index has only 8 entries).

Per core, everything runs in bf16 (rel err ~3e-3 vs the 2e-2 gate): the
whole 16 MiB x^T stays resident in SBUF so W streams from HBM exactly
once (32 MiB bf16) in host-pretransposed panels that are fully contiguous
per partition; the output is written back as bf16 (host casts to f32).
The LoRA delta and bias fold into the same PSUM accumulation group as
the base matmul via one extra K=33 matmul (32 axT rows + a ones row
paired with [0.5*B^T; bias]).

Measured on HW: matmuls accumulating into a single PSUM bank sustain
~79 ns (N=512 bf16) while interleaving 8 banks per k-step costs ~125 ns,
so the inner loop runs bank-SEQUENTIAL: each 128x512 output tile does
its full 33-step accumulation in one bank, banks rotate per tile, and
evictions (alternating vector/scalar engines) overlap the next 7 tiles.
"""
import sys
sys.path.insert(0, "/opt/trn_rl_repo")
import numpy as np

B, S, DIN, DOUT, R = 8, 2048, 4096, 4096, 32
LORA_SCALING = 16 / 32
KT = DIN // 128          # 32 contraction tiles
SB = S // 128            # 16 s-blocks
OB = DOUT // 512         # 8 output panels
HPK = 16                 # k-tiles per W half-panel
N_CORES = 8

_cache = {}


def _build(hw_loop=1):
    import concourse.bacc as bacc
    import concourse.mybir as mybir
    from concourse.tile import TileContext

    BF16 = mybir.dt.bfloat16
    F32 = mybir.dt.float32

    nc = bacc.Bacc()
    xT = nc.dram_tensor("xT", [128, KT * S], BF16, kind="ExternalInput")
    WT = nc.dram_tensor("WT", [128, OB * KT * 512], BF16, kind="ExternalInput")
    AbR = nc.dram_tensor("AbR", [128, KT * R], BF16, kind="ExternalInput")
    # rows 0-31: 0.5*lora_B.T, row 32: bias
    BTa = nc.dram_tensor("BTa", [R + 1, DOUT], BF16, kind="ExternalInput")
    ONES = nc.dram_tensor("ONES", [1, 512], BF16, kind="ExternalInput")
    out = nc.dram_tensor("out", [S, DOUT], BF16, kind="ExternalOutput")

    with TileContext(nc) as tc:
        with (
            tc.tile_pool(name="xp", bufs=KT) as xp,
            tc.tile_pool(name="wp", bufs=2) as wp,
            tc.tile_pool(name="cp", bufs=1) as cp,
            tc.tile_pool(name="axp", bufs=4) as axp,
            tc.tile_pool(name="op", bufs=8) as op_,
            tc.tile_pool(name="pp", bufs=1, space="PSUM") as pp,
        ):
            ab = cp.tile([128, KT * R], BF16, name="ab")
            nc.sync.dma_start(ab, AbR[:, :])
            bt = cp.tile([R + 1, DOUT], BF16, name="bt")
            nc.sync.dma_start(bt, BTa[:, :])

            def body():
                xt = []
                for k in range(KT):
                    t = xp.tile([128, S], BF16, name="xq")
                    nc.sync.dma_start(t, xT[:, k * S:(k + 1) * S])
                    xt.append(t)
                # axT (transposed lora activations + ones row) per 512 cols
                axc = []
                for c in range(S // 512):
                    pa = pp.tile([R, 512], F32, name=f"ps{c}")
                    for k in range(KT):
                        nc.tensor.matmul(
                            pa, lhsT=ab[:, k * R:(k + 1) * R],
                            rhs=xt[k][:, c * 512:(c + 1) * 512],
                            start=(k == 0), stop=(k == KT - 1))
                    axt = axp.tile([R + 1, 512], BF16, name="axt")
                    nc.vector.tensor_copy(axt[0:R, :], pa)
                    nc.sync.dma_start(axt[R:R + 1, :], ONES[0:1, :])
                    axc.append(axt)
                # main panels: base matmul + fused lora delta + bias.
                # bank-sequential: one 128x512 tile accumulates 33 steps in
                # one PSUM bank; banks rotate per tile so evictions overlap.
                for ob in range(OB):
                    hp = []
                    for i in range(KT // HPK):
                        w = wp.tile([128, HPK * 512], BF16, name="wt")
                        off = (ob * KT + i * HPK) * 512
                        nc.sync.dma_start(w, WT[:, off:off + HPK * 512])
                        hp.append(w)
                    for sbg in range(SB):
                        ps = pp.tile([128, 512], F32, name=f"ps{sbg % 8}")
                        col = sbg * 128
                        for k in range(KT):
                            nc.tensor.matmul(
                                ps, lhsT=xt[k][:, col:col + 128],
                                rhs=hp[k // HPK][
                                    :, (k % HPK) * 512:(k % HPK + 1) * 512],
                                start=(k == 0), stop=False)
                        nc.tensor.matmul(
                            ps,
                            lhsT=axc[sbg // 4][
                                :, (sbg % 4) * 128:(sbg % 4 + 1) * 128],
                            rhs=bt[:, ob * 512:(ob + 1) * 512],
                            start=False, stop=True)
                        ot = op_.tile([128, 512], BF16, name="ot")
                        if sbg % 2 == 0:
                            nc.vector.tensor_copy(ot, ps)
                        else:
                            nc.scalar.copy(ot, ps)
                        nc.sync.dma_start(
                            out[col:col + 128, ob * 512:(ob + 1) * 512], ot)

            if hw_loop > 1:
                with tc.For_i(0, hw_loop, 1):
                    body()
            else:
                body()
    nc.finalize()
    return nc


def _prep_in_maps(input, weight, bias, lora_A, lora_B, labeler_index):
    import ml_dtypes
    bf16 = ml_dtypes.bfloat16

    x = np.asarray(input, dtype=np.float32)
    W = np.asarray(weight, dtype=np.float32)
    bias = np.asarray(bias, dtype=np.float32)
    lA = np.asarray(lora_A, dtype=np.float32)
    lB = np.asarray(lora_B, dtype=np.float32)
    idx = np.asarray(labeler_index).astype(np.int64)

    # W^T tiled as [128, OB, KT, 512] so a half-panel DMA is contiguous
    WTr = np.ascontiguousarray(
        W.T.reshape(KT, 128, OB, 512).transpose(1, 2, 0, 3)
    ).astype(bf16).reshape(128, OB * KT * 512)
    BTa = np.concatenate(
        [LORA_SCALING * lB.T, bias[None, :]], axis=0).astype(bf16)
    ones = np.ones((1, 512), dtype=bf16)

    in_maps = []
    for b in range(B):
        xTr = np.ascontiguousarray(
            x[b].T.reshape(KT, 128, S).transpose(1, 0, 2)
        ).astype(bf16).reshape(128, KT * S)
        AbR = np.ascontiguousarray(
            lA[idx[b]].reshape(KT, 128, R).transpose(1, 0, 2)
        ).astype(bf16).reshape(128, KT * R)
        in_maps.append({"xT": xTr, "WT": WTr, "AbR": AbR, "BTa": BTa,
                        "ONES": ones})
    return in_maps


def kernel(input, weight, bias, lora_A, lora_B, labeler_index):
    from concourse import bass_utils

    in_maps = _prep_in_maps(input, weight, bias, lora_A, lora_B, labeler_index)
    if "nc" not in _cache:
        _cache["nc"] = _build()
    last_err = None
    for attempt in range(3):
        try:
            res = bass_utils.run_bass_kernel_spmd(
                _cache["nc"], in_maps, core_ids=list(range(N_CORES)))
            return np.stack([res.results[b]["out"].astype(np.float32)
                             for b in range(B)])
        except Exception as e:  # transient NRT wedge from a prior crashed run
            last_err = e
            if "UNRECOVERABLE" not in str(e) and "UNAVAILABLE" not in str(e):
                raise
    raise last_err
